# revision 2
# baseline (speedup 1.0000x reference)
"""Trainium2 Bass kernel for AttributionCentroidTracker.

Reference computation (B=512, V=32768, C=16):
    Wg[b, v]   = W_eff[b, v, labels[b]]
    attr[b, v] = |sparse_vector[b, v] * Wg[b, v]|
    sums[c, v] = segment_sum(attr, labels)       # [C, V]
    mean       = sums / max(counts, 1)
    out[c]     = centroids[c]                     if counts[c] == 0
               = mean[c]                          if not initialized[c]
               = M*centroids[c] + (1-M)*mean[c]   otherwise

Device strategy (8 cores, sharded along V — per-class sums are complete
locally per V-slice, so no cross-core reduction is needed):
  - b (512) lives on the 128 SBUF partitions in 4 groups of 128.
  - Stream W in [128, 4*VC*16] tiles; ScalarE takes |W| in place, VectorE
    multiplies by |sv| broadcast over the class axis (stride-0 AP).
  - Scatter/segment-sum via TensorE: for each (class c, batch-group g) a
    masked one-hot lhsT [128, 16] (only column c nonzero) contracts the
    batch partitions against the stride-16 class-c slice of the tile,
    accumulating all 64 matmuls into one [16, VC] PSUM tile.
  - ScalarE evacuates PSUM scaled by the per-class update coefficient a_c;
    out = a_c * sums + b_c * centroids with a/b computed on host from
    labels/counts/initialized (tiny [16] inputs - program stays generic).
"""

import os
import sys

import numpy as np

if "/opt/trn_rl_repo" not in sys.path:
    sys.path.insert(0, "/opt/trn_rl_repo")

B, V, C = 512, 32768, 16
NCORES = 8
VSH = V // NCORES            # 4096 columns of V per core
P = 128                      # SBUF partitions
BG = B // P                  # 4 batch groups
VC = 128                     # v-chunk per tile
NVC = VSH // VC              # 32 tiles per core
STEPS_PER_EPOCH = 1000
MOMENTUM = 1.0 - 2.0 / (STEPS_PER_EPOCH + 1)

_CACHE = {}

last_exec_time_ns = None
last_results = None


def _build_nc():
    import concourse.bacc as bacc
    import concourse.tile as tile
    from concourse import mybir

    f32 = mybir.dt.float32
    nc = bacc.Bacc("TRN2", target_bir_lowering=False, debug=False)

    w = nc.dram_tensor("w", [B, VSH, C], f32, kind="ExternalInput")
    sv = nc.dram_tensor("sv", [B, VSH], f32, kind="ExternalInput")
    ohm = nc.dram_tensor("ohm", [P, C * BG * C], f32, kind="ExternalInput")
    cent = nc.dram_tensor("cent", [C, VSH], f32, kind="ExternalInput")
    avec = nc.dram_tensor("avec", [C, 1], f32, kind="ExternalInput")
    bvec = nc.dram_tensor("bvec", [C, 1], f32, kind="ExternalInput")
    out = nc.dram_tensor("out", [C, VSH], f32, kind="ExternalOutput")

    # b = g*128 + p  ->  partition p, group g
    w_r = w.ap().rearrange("(g p) v c -> p g v c", p=P)      # [128, 4, VSH, 16]
    sv_r = sv.ap().rearrange("(g p) v -> p g v", p=P)        # [128, 4, VSH]

    with tile.TileContext(nc) as tc:
        with (
            tc.tile_pool(name="const", bufs=1) as cpool,
            tc.tile_pool(name="wp", bufs=4) as wpool,
            tc.tile_pool(name="svp", bufs=4) as svpool,
            tc.tile_pool(name="evp", bufs=2) as evpool,
            tc.tile_pool(name="psum", bufs=2, space="PSUM") as ppool,
        ):
            ohm_sb = cpool.tile([P, C * BG * C], dtype=f32)
            nc.sync.dma_start(out=ohm_sb[:], in_=ohm.ap())
            cent_sb = cpool.tile([C, VSH], dtype=f32)
            nc.sync.dma_start(out=cent_sb[:], in_=cent.ap())
            avec_sb = cpool.tile([C, 1], dtype=f32)
            nc.sync.dma_start(out=avec_sb[:], in_=avec.ap())
            bvec_sb = cpool.tile([C, 1], dtype=f32)
            nc.sync.dma_start(out=bvec_sb[:], in_=bvec.ap())

            out_sb = cpool.tile([C, VSH], dtype=f32)
            # out = b_c * centroids, then the per-chunk a_c * sums is added.
            nc.vector.tensor_scalar_mul(out_sb[:], cent_sb[:], bvec_sb[:])

            for i in range(NVC):
                vlo = i * VC
                wt = wpool.tile([P, BG * VC * C], dtype=f32, tag="wt")
                wt4 = wt[:].rearrange("p (g v c) -> p g v c", g=BG, v=VC)
                nc.sync.dma_start(out=wt4, in_=w_r[:, :, vlo : vlo + VC, :])

                svt = svpool.tile([P, BG * VC], dtype=f32, tag="svt")
                svt3 = svt[:].rearrange("p (g v) -> p g v", g=BG)
                nc.sync.dma_start(out=svt3, in_=sv_r[:, :, vlo : vlo + VC])

                nc.scalar.activation(
                    wt[:], wt[:], mybir.ActivationFunctionType.Abs
                )
                nc.scalar.activation(
                    svt[:], svt[:], mybir.ActivationFunctionType.Abs
                )
                nc.vector.tensor_tensor(
                    out=wt4,
                    in0=wt4,
                    in1=svt3.to_broadcast([P, BG, VC, C]),
                    op=mybir.AluOpType.mult,
                )

                ps = ppool.tile([C, VC], dtype=f32, tag="ps")
                n = 0
                for c in range(C):
                    for g in range(BG):
                        nc.tensor.matmul(
                            out=ps[:],
                            lhsT=ohm_sb[:, (c * BG + g) * C : (c * BG + g + 1) * C],
                            rhs=wt4[:, g, :, c],
                            start=(n == 0),
                            stop=(n == C * BG - 1),
                        )
                        n += 1

                ev = evpool.tile([C, VC], dtype=f32, tag="ev")
                nc.scalar.activation(
                    ev[:],
                    ps[:],
                    mybir.ActivationFunctionType.Copy,
                    bias=0.0,
                    scale=avec_sb[:],
                )
                nc.vector.tensor_tensor(
                    out=out_sb[:, vlo : vlo + VC],
                    in0=out_sb[:, vlo : vlo + VC],
                    in1=ev[:],
                    op=mybir.AluOpType.add,
                )

            nc.sync.dma_start(out=out.ap(), in_=out_sb[:])

    nc.finalize()
    return nc


def _get_nc():
    if "nc" not in _CACHE:
        _CACHE["nc"] = _build_nc()
    return _CACHE["nc"]


def kernel(sparse_vector, W_eff, labels, centroids, initialized):
    global last_exec_time_ns, last_results
    from concourse.bass_utils import run_bass_kernel_spmd

    sv = np.ascontiguousarray(np.asarray(sparse_vector, dtype=np.float32))
    w = np.asarray(W_eff, dtype=np.float32)
    lab = np.asarray(labels).astype(np.int64)
    cent = np.ascontiguousarray(np.asarray(centroids, dtype=np.float32))
    init = np.asarray(initialized).astype(bool)

    # Host-side label-derived constants (tiny) — keep the program generic.
    oh = lab[:, None] == np.arange(C)[None, :]           # [B, C] bool
    counts = oh.sum(axis=0).astype(np.float64)           # [C]
    present = counts > 0
    safe = np.maximum(counts, 1.0)
    a = np.where(present, np.where(init, (1.0 - MOMENTUM) / safe, 1.0 / safe), 0.0)
    b = np.where(present, np.where(init, MOMENTUM, 0.0), 1.0)
    avec = a.astype(np.float32).reshape(C, 1)
    bvec = b.astype(np.float32).reshape(C, 1)

    # Masked one-hot lhsT blocks: ohm[p, (c*BG+g)*C + c] = 1 iff labels[g*128+p]==c
    lab2 = lab.reshape(BG, P)                            # [g, p]
    ohm = np.zeros((P, C * BG * C), np.float32)
    for c in range(C):
        for g in range(BG):
            ohm[:, (c * BG + g) * C + c] = (lab2[g] == c).astype(np.float32)

    nc = _get_nc()
    in_maps = []
    for i in range(NCORES):
        s = i * VSH
        in_maps.append(
            {
                "w": np.ascontiguousarray(w[:, s : s + VSH, :]),
                "sv": np.ascontiguousarray(sv[:, s : s + VSH]),
                "ohm": ohm,
                "cent": np.ascontiguousarray(cent[:, s : s + VSH]),
                "avec": avec,
                "bvec": bvec,
            }
        )

    res = run_bass_kernel_spmd(nc, in_maps, core_ids=list(range(NCORES)))
    last_exec_time_ns = res.exec_time_ns
    last_results = res
    return np.concatenate([res.results[i]["out"] for i in range(NCORES)], axis=1)


# revision 4
# speedup vs baseline: 1.7992x; 1.7992x over previous
"""Trainium2 Bass kernel for AttributionCentroidTracker.

Reference computation (B=512, V=32768, C=16):
    Wg[b, v]   = W_eff[b, v, labels[b]]
    attr[b, v] = |sparse_vector[b, v] * Wg[b, v]|
    sums[c, v] = segment_sum(attr, labels)       # [C, V]
    mean       = sums / max(counts, 1)
    out[c]     = centroids[c]                     if counts[c] == 0
               = mean[c]                          if not initialized[c]
               = M*centroids[c] + (1-M)*mean[c]   otherwise

Device strategy (8 cores, sharded along V — per-class sums are complete
locally per V-slice, so no cross-core reduction is needed):
  - b (512) lives on the 128 SBUF partitions in 4 groups of 128.
  - Stream W in [128, 4*VC*16] tiles; ScalarE takes |W| in place, VectorE
    multiplies by |sv| broadcast over the class axis (stride-0 AP).
  - Scatter/segment-sum via TensorE: for each (class c, batch-group g) a
    masked one-hot lhsT [128, 16] (only column c nonzero) contracts the
    batch partitions against the stride-16 class-c slice of the tile,
    accumulating all 64 matmuls into one [16, VC] PSUM tile.
  - ScalarE evacuates PSUM scaled by the per-class update coefficient a_c;
    out = a_c * sums + b_c * centroids with a/b computed on host from
    labels/counts/initialized (tiny [16] inputs - program stays generic).
"""

import os
import sys

import numpy as np

if "/opt/trn_rl_repo" not in sys.path:
    sys.path.insert(0, "/opt/trn_rl_repo")

B, V, C = 512, 32768, 16
NCORES = 8
VSH = V // NCORES            # 4096 columns of V per core
P = 128                      # SBUF partitions
BG = B // P                  # 4 batch groups
VC = 128                     # v-chunk per tile
NVC = VSH // VC              # 32 tiles per core
STEPS_PER_EPOCH = 1000
MOMENTUM = 1.0 - 2.0 / (STEPS_PER_EPOCH + 1)

_CACHE = {}

last_exec_time_ns = None
last_results = None


def _build_nc():
    import concourse.bacc as bacc
    import concourse.tile as tile
    from concourse import mybir

    f32 = mybir.dt.float32
    nc = bacc.Bacc("TRN2", target_bir_lowering=False, debug=False)

    w = nc.dram_tensor("w", [B, VSH, C], f32, kind="ExternalInput")
    sv = nc.dram_tensor("sv", [B, VSH], f32, kind="ExternalInput")
    ohm = nc.dram_tensor("ohm", [P, C * BG * C], f32, kind="ExternalInput")
    cent = nc.dram_tensor("cent", [C, VSH], f32, kind="ExternalInput")
    avec = nc.dram_tensor("avec", [C, 1], f32, kind="ExternalInput")
    bvec = nc.dram_tensor("bvec", [C, 1], f32, kind="ExternalInput")
    out = nc.dram_tensor("out", [C, VSH], f32, kind="ExternalOutput")

    # b = g*128 + p  ->  partition p, group g
    w_r = w.ap().rearrange("(g p) v c -> p g v c", p=P)      # [128, 4, VSH, 16]
    sv_r = sv.ap().rearrange("(g p) v -> p g v", p=P)        # [128, 4, VSH]

    with tile.TileContext(nc) as tc:
        with (
            tc.tile_pool(name="const", bufs=1) as cpool,
            tc.tile_pool(name="wp", bufs=3) as wpool,
            tc.tile_pool(name="yp", bufs=2) as ypool,
            tc.tile_pool(name="svp", bufs=4) as svpool,
            tc.tile_pool(name="evp", bufs=2) as evpool,
            tc.tile_pool(name="psum", bufs=2, space="PSUM") as ppool,
        ):
            ohm_sb = cpool.tile([P, C * BG * C], dtype=f32)
            nc.sync.dma_start(out=ohm_sb[:], in_=ohm.ap())
            cent_sb = cpool.tile([C, VSH], dtype=f32)
            nc.sync.dma_start(out=cent_sb[:], in_=cent.ap())
            avec_sb = cpool.tile([C, 1], dtype=f32)
            nc.sync.dma_start(out=avec_sb[:], in_=avec.ap())
            bvec_sb = cpool.tile([C, 1], dtype=f32)
            nc.sync.dma_start(out=bvec_sb[:], in_=bvec.ap())

            out_sb = cpool.tile([C, VSH], dtype=f32)
            # out = b_c * centroids, then the per-chunk a_c * sums is added.
            nc.vector.tensor_scalar_mul(out_sb[:], cent_sb[:], bvec_sb[:])

            for i in range(NVC):
                vlo = i * VC
                wt = wpool.tile([P, BG * VC * C], dtype=f32, tag="wt")
                wt4 = wt[:].rearrange("p (g v c) -> p g v c", g=BG, v=VC)
                nc.sync.dma_start(out=wt4, in_=w_r[:, :, vlo : vlo + VC, :])

                svt = svpool.tile([P, BG * VC], dtype=f32, tag="svt")
                svt3 = svt[:].rearrange("p (g v) -> p g v", g=BG)
                nc.sync.dma_start(out=svt3, in_=sv_r[:, :, vlo : vlo + VC])

                nc.scalar.activation(
                    wt[:], wt[:], mybir.ActivationFunctionType.Abs
                )
                nc.scalar.activation(
                    svt[:], svt[:], mybir.ActivationFunctionType.Abs
                )
                # Y is written c-major: yt[p, g, c, v] = |W[p,g,v,c]|*|sv[p,g,v]|
                # so each class's matmul rhs is a contiguous VC-long slice
                # (strided PE rhs reads were the bottleneck: ~4x column rate).
                yt = ypool.tile([P, BG * C * VC], dtype=f32, tag="yt")
                yt_w = yt[:].rearrange("p (g c v) -> p g v c", g=BG, c=C, v=VC)
                nc.vector.tensor_tensor(
                    out=yt_w,
                    in0=wt4,
                    in1=svt3.to_broadcast([P, BG, VC, C]),
                    op=mybir.AluOpType.mult,
                )
                yt_r = yt[:].rearrange("p (g c v) -> p g c v", g=BG, c=C, v=VC)

                ps = ppool.tile([C, VC], dtype=f32, tag="ps")
                n = 0
                for c in range(C):
                    for g in range(BG):
                        nc.tensor.matmul(
                            out=ps[:],
                            lhsT=ohm_sb[:, (c * BG + g) * C : (c * BG + g + 1) * C],
                            rhs=yt_r[:, g, c, :],
                            start=(n == 0),
                            stop=(n == C * BG - 1),
                        )
                        n += 1

                ev = evpool.tile([C, VC], dtype=f32, tag="ev")
                nc.scalar.activation(
                    ev[:],
                    ps[:],
                    mybir.ActivationFunctionType.Copy,
                    bias=0.0,
                    scale=avec_sb[:],
                )
                nc.vector.tensor_tensor(
                    out=out_sb[:, vlo : vlo + VC],
                    in0=out_sb[:, vlo : vlo + VC],
                    in1=ev[:],
                    op=mybir.AluOpType.add,
                )

            nc.sync.dma_start(out=out.ap(), in_=out_sb[:])

    nc.finalize()
    return nc


def _get_nc():
    if "nc" not in _CACHE:
        _CACHE["nc"] = _build_nc()
    return _CACHE["nc"]


def kernel(sparse_vector, W_eff, labels, centroids, initialized):
    global last_exec_time_ns, last_results
    from concourse.bass_utils import run_bass_kernel_spmd

    sv = np.ascontiguousarray(np.asarray(sparse_vector, dtype=np.float32))
    w = np.asarray(W_eff, dtype=np.float32)
    lab = np.asarray(labels).astype(np.int64)
    cent = np.ascontiguousarray(np.asarray(centroids, dtype=np.float32))
    init = np.asarray(initialized).astype(bool)

    # Host-side label-derived constants (tiny) — keep the program generic.
    oh = lab[:, None] == np.arange(C)[None, :]           # [B, C] bool
    counts = oh.sum(axis=0).astype(np.float64)           # [C]
    present = counts > 0
    safe = np.maximum(counts, 1.0)
    a = np.where(present, np.where(init, (1.0 - MOMENTUM) / safe, 1.0 / safe), 0.0)
    b = np.where(present, np.where(init, MOMENTUM, 0.0), 1.0)
    avec = a.astype(np.float32).reshape(C, 1)
    bvec = b.astype(np.float32).reshape(C, 1)

    # Masked one-hot lhsT blocks: ohm[p, (c*BG+g)*C + c] = 1 iff labels[g*128+p]==c
    lab2 = lab.reshape(BG, P)                            # [g, p]
    ohm = np.zeros((P, C * BG * C), np.float32)
    for c in range(C):
        for g in range(BG):
            ohm[:, (c * BG + g) * C + c] = (lab2[g] == c).astype(np.float32)

    nc = _get_nc()
    in_maps = []
    for i in range(NCORES):
        s = i * VSH
        in_maps.append(
            {
                "w": np.ascontiguousarray(w[:, s : s + VSH, :]),
                "sv": np.ascontiguousarray(sv[:, s : s + VSH]),
                "ohm": ohm,
                "cent": np.ascontiguousarray(cent[:, s : s + VSH]),
                "avec": avec,
                "bvec": bvec,
            }
        )

    res = run_bass_kernel_spmd(nc, in_maps, core_ids=list(range(NCORES)))
    last_exec_time_ns = res.exec_time_ns
    last_results = res
    return np.concatenate([res.results[i]["out"] for i in range(NCORES)], axis=1)


# revision 5
# speedup vs baseline: 1.9284x; 1.0718x over previous
"""Trainium2 Bass kernel for AttributionCentroidTracker.

Reference computation (B=512, V=32768, C=16):
    Wg[b, v]   = W_eff[b, v, labels[b]]
    attr[b, v] = |sparse_vector[b, v] * Wg[b, v]|
    sums[c, v] = segment_sum(attr, labels)       # [C, V]
    mean       = sums / max(counts, 1)
    out[c]     = centroids[c]                     if counts[c] == 0
               = mean[c]                          if not initialized[c]
               = M*centroids[c] + (1-M)*mean[c]   otherwise

Device strategy (8 cores, sharded along V — per-class sums are complete
locally per V-slice, so no cross-core reduction is needed):
  - b (512) lives on the 128 SBUF partitions in 4 groups of 128.
  - W streams in as bf16 (SWDGE cast-DMA) [128, 4*VC*16] tiles; ScalarE
    takes |W| in place, VectorE multiplies by |sv| (stride-0 broadcast).
  - Scatter/segment-sum on TensorE: per (class, batch-group) a masked
    one-hot lhsT [128, 16] (only column c nonzero) contracts the batch
    partitions against the class-c slice of Y, all 64 matmuls
    accumulating into one fp32 [16, VC] PSUM tile.
  - The class-c slice is stride-16 in natural (v,c) layout, which costs
    the PE ~4x column rate; a c-major copy costs the DVE ~1.7x. Neither
    engine can absorb the full permute under the DMA roofline, so PERM
    out of every 3 tiles are permuted to c-major during the multiply
    (strided-read TT) and the rest keep natural layout with strided-rhs
    matmuls — balancing DVE and PE both below the DMA bound.
  - ScalarE evacuates PSUM scaled by the per-class coefficient a_c;
    out = a_c * sums + b_c * centroids with a/b computed on host from
    labels/counts/initialized (tiny [16] inputs - program stays generic).
"""

import os
import sys

import numpy as np

if "/opt/trn_rl_repo" not in sys.path:
    sys.path.insert(0, "/opt/trn_rl_repo")

B, V, C = 512, 32768, 16
NCORES = 8
VSH = V // NCORES            # 4096 columns of V per core
P = 128                      # SBUF partitions
BG = B // P                  # 4 batch groups
VC = 128                     # v-chunk per tile
NVC = VSH // VC              # 32 tiles per core
PERM_EVERY = 3               # 1 of every 3 tiles gets the c-major permute
STEPS_PER_EPOCH = 1000
MOMENTUM = 1.0 - 2.0 / (STEPS_PER_EPOCH + 1)

_CACHE = {}

last_exec_time_ns = None
last_results = None


def _build_nc():
    import concourse.bacc as bacc
    import concourse.tile as tile
    from concourse import mybir

    f32 = mybir.dt.float32
    bf16 = mybir.dt.bfloat16
    Abs = mybir.ActivationFunctionType.Abs
    nc = bacc.Bacc("TRN2", target_bir_lowering=False, debug=False)

    w = nc.dram_tensor("w", [B, VSH, C], f32, kind="ExternalInput")
    sv = nc.dram_tensor("sv", [B, VSH], f32, kind="ExternalInput")
    ohm = nc.dram_tensor("ohm", [P, C * BG * C], bf16, kind="ExternalInput")
    cent = nc.dram_tensor("cent", [C, VSH], f32, kind="ExternalInput")
    avec = nc.dram_tensor("avec", [C, 1], f32, kind="ExternalInput")
    bvec = nc.dram_tensor("bvec", [C, 1], f32, kind="ExternalInput")
    out = nc.dram_tensor("out", [C, VSH], f32, kind="ExternalOutput")

    # b = g*128 + p  ->  partition p, group g
    w_r = w.ap().rearrange("(g p) v c -> p g v c", p=P)      # [128, 4, VSH, 16]
    sv_r = sv.ap().rearrange("(g p) v -> p g v", p=P)        # [128, 4, VSH]

    with tile.TileContext(nc) as tc:
        with (
            tc.tile_pool(name="const", bufs=1) as cpool,
            tc.tile_pool(name="wp", bufs=4) as wpool,
            tc.tile_pool(name="yp", bufs=3) as ypool,
            tc.tile_pool(name="evp", bufs=2) as evpool,
            tc.tile_pool(name="psum", bufs=2, space="PSUM") as ppool,
        ):
            ohm_sb = cpool.tile([P, C * BG * C], dtype=bf16)
            nc.sync.dma_start(out=ohm_sb[:], in_=ohm.ap())
            cent_sb = cpool.tile([C, VSH], dtype=f32)
            nc.sync.dma_start(out=cent_sb[:], in_=cent.ap())
            avec_sb = cpool.tile([C, 1], dtype=f32)
            nc.sync.dma_start(out=avec_sb[:], in_=avec.ap())
            bvec_sb = cpool.tile([C, 1], dtype=f32)
            nc.sync.dma_start(out=bvec_sb[:], in_=bvec.ap())

            # |sv| as bf16, all batch groups, whole core V-slice: 4 MB
            svt = cpool.tile([P, BG * VSH], dtype=bf16)
            svt3 = svt[:].rearrange("p (g v) -> p g v", g=BG)
            nc.gpsimd.dma_start(out=svt3, in_=sv_r)          # f32 -> bf16 cast
            nc.scalar.activation(svt[:], svt[:], Abs)

            out_sb = cpool.tile([C, VSH], dtype=f32)
            # out = b_c * centroids; per-chunk a_c * sums is added later.
            nc.vector.tensor_scalar_mul(out_sb[:], cent_sb[:], bvec_sb[:])

            for i in range(NVC):
                vlo = i * VC
                perm = (i % PERM_EVERY) == 0

                wt = wpool.tile([P, BG * VC * C], dtype=bf16, tag="wt")
                wt4 = wt[:].rearrange("p (g v c) -> p g v c", g=BG, v=VC)
                nc.gpsimd.dma_start(out=wt4, in_=w_r[:, :, vlo : vlo + VC, :])
                nc.scalar.activation(wt[:], wt[:], Abs)

                if perm:
                    # Multiply with strided in0 (c-major view of natural
                    # layout), contiguous c-major output in a fresh tile.
                    yt = ypool.tile([P, BG * C * VC], dtype=bf16, tag="yt")
                    y4 = yt[:].rearrange("p (g c v) -> p g c v", g=BG, c=C)
                    in0 = wt[:].rearrange("p (g v c) -> p g c v", g=BG, v=VC)
                    in1 = (
                        svt3[:, :, vlo : vlo + VC]
                        .unsqueeze(2)
                        .broadcast_to([P, BG, C, VC])
                    )
                    nc.vector.tensor_tensor(
                        out=y4, in0=in0, in1=in1, op=mybir.AluOpType.mult
                    )
                    rhs_fn = lambda g, c: y4[:, g, c, :]
                else:
                    # In-place multiply, natural layout; PE pays the stride.
                    in1 = svt3[:, :, vlo : vlo + VC].to_broadcast(
                        [P, BG, VC, C]
                    )
                    nc.vector.tensor_tensor(
                        out=wt4, in0=wt4, in1=in1, op=mybir.AluOpType.mult
                    )
                    rhs_fn = lambda g, c: wt4[:, g, :, c]

                ps = ppool.tile([C, VC], dtype=f32, tag="ps")
                n = 0
                for c in range(C):
                    for g in range(BG):
                        nc.tensor.matmul(
                            out=ps[:],
                            lhsT=ohm_sb[:, (c * BG + g) * C : (c * BG + g + 1) * C],
                            rhs=rhs_fn(g, c),
                            start=(n == 0),
                            stop=(n == C * BG - 1),
                        )
                        n += 1

                ev = evpool.tile([C, VC], dtype=f32, tag="ev")
                nc.scalar.activation(
                    ev[:],
                    ps[:],
                    mybir.ActivationFunctionType.Copy,
                    bias=0.0,
                    scale=avec_sb[:],
                )
                nc.vector.tensor_tensor(
                    out=out_sb[:, vlo : vlo + VC],
                    in0=out_sb[:, vlo : vlo + VC],
                    in1=ev[:],
                    op=mybir.AluOpType.add,
                )

            nc.sync.dma_start(out=out.ap(), in_=out_sb[:])

    nc.finalize()
    return nc


def _get_nc():
    if "nc" not in _CACHE:
        _CACHE["nc"] = _build_nc()
    return _CACHE["nc"]


def kernel(sparse_vector, W_eff, labels, centroids, initialized):
    global last_exec_time_ns, last_results
    import ml_dtypes
    from concourse.bass_utils import run_bass_kernel_spmd

    sv = np.ascontiguousarray(np.asarray(sparse_vector, dtype=np.float32))
    w = np.asarray(W_eff, dtype=np.float32)
    lab = np.asarray(labels).astype(np.int64)
    cent = np.ascontiguousarray(np.asarray(centroids, dtype=np.float32))
    init = np.asarray(initialized).astype(bool)

    # Host-side label-derived constants (tiny) — keep the program generic.
    oh = lab[:, None] == np.arange(C)[None, :]           # [B, C] bool
    counts = oh.sum(axis=0).astype(np.float64)           # [C]
    present = counts > 0
    safe = np.maximum(counts, 1.0)
    a = np.where(present, np.where(init, (1.0 - MOMENTUM) / safe, 1.0 / safe), 0.0)
    b = np.where(present, np.where(init, MOMENTUM, 0.0), 1.0)
    avec = a.astype(np.float32).reshape(C, 1)
    bvec = b.astype(np.float32).reshape(C, 1)

    # Masked one-hot lhsT blocks: ohm[p, (c*BG+g)*C + c] = 1 iff labels[g*128+p]==c
    lab2 = lab.reshape(BG, P)                            # [g, p]
    ohm = np.zeros((P, C * BG * C), np.float32)
    for c in range(C):
        for g in range(BG):
            ohm[:, (c * BG + g) * C + c] = (lab2[g] == c).astype(np.float32)
    ohm = ohm.astype(ml_dtypes.bfloat16)

    nc = _get_nc()
    in_maps = []
    for i in range(NCORES):
        s = i * VSH
        in_maps.append(
            {
                "w": np.ascontiguousarray(w[:, s : s + VSH, :]),
                "sv": np.ascontiguousarray(sv[:, s : s + VSH]),
                "ohm": ohm,
                "cent": np.ascontiguousarray(cent[:, s : s + VSH]),
                "avec": avec,
                "bvec": bvec,
            }
        )

    res = run_bass_kernel_spmd(nc, in_maps, core_ids=list(range(NCORES)))
    last_exec_time_ns = res.exec_time_ns
    last_results = res
    return np.concatenate([res.results[i]["out"] for i in range(NCORES)], axis=1)
